# revision 35
# baseline (speedup 1.0000x reference)
"""Trainium2 Bass kernel for nn_Attention_25572235280790.

Dense attention block (B=16, C=256, H=W=32, NH=8, HD=32) with RoPE-style theta
shift, LePE depthwise 5x5 conv, BN+SiLU gate, channel LayerNorms and 1x1 convs.

Sharding: data-parallel over batch across 8 NeuronCores (2 batches/core), no
collectives.  Inside each core everything is computed in two layouts:
  - [c, l]  (channels on partitions)  for the 1x1 convs / scores / lepe
  - [l, c]  (spatial on partitions)   for softmax-normalize / layernorms / gate

v2 layout: q/k packed 4 heads per 128-partition tile (offsets 0/32/64/96).
Scores S^T[m, l] for the 4 heads of a group are computed CONCURRENTLY with
row-tiled matmuls (tile_position=(32i, 0), K=32 each) into one 4-bank PSUM
tile, and evacuated by a single merged exp ACT (N=2048) on ScalarE -- the
bottleneck engine.  PV accumulates all 4 heads of an l-tile into one PSUM
bank; softmax-normalize reads PSUM directly (no copy), dividing by the
ones-augmented V column sum.  LN rsqrt is computed on VectorE via the quake
bit-trick + 2 Newton steps, so ScalarE never leaves the exp/tanh table set.
"""

import numpy as np
import ml_dtypes

import concourse.bass as bass
import concourse.tile as tile
from concourse import bacc
from concourse import mybir
from concourse.alu_op_type import AluOpType

B, C, H, W = 16, 256, 32, 32
NH, HD = 8, 32
SCALE = HD ** -0.5
LN_EPS = 1e-6
BN_EPS = 1e-5
L = H * W
NCORES = 8
BPC = B // NCORES          # batches per core
AF = mybir.ActivationFunctionType
F32 = mybir.dt.float32
BF16 = mybir.dt.bfloat16
I32 = mybir.dt.int32

NPBF = ml_dtypes.bfloat16
# es tiles are fp8-e3m4: softmax is shift-invariant, so exp(s*SCALE - 0.5)
# keeps the observed range [0.057, 20.9] inside e3m4's [~0.008, 15.5] while
# e3m4's 4 mantissa bits halve the quantization error of e4m3.
ES_DT = mybir.dt.float8e3
ES_SHIFT = -0.5
ES_BUFS = 20


def build_program():
    nc = bacc.Bacc()
    dp = nc.declare_dram_parameter
    io = {
        'x2':      dp('x2',      [BPC, C, L],   BF16, isOutput=False),
        'wqkT':    dp('wqkT',    [C, 512],      BF16, isOutput=False),
        'bqk':     dp('bqk',     [128, 4],      F32,  isOutput=False),
        'wvT':     dp('wvT',     [C, C],        BF16, isOutput=False),
        'bv':      dp('bv',      [128, 2],      F32,  isOutput=False),
        'rhsvg':   dp('rhsvg',   [C, 512],      BF16, isOutput=False),
        'bvgbc':   dp('bvgbc',   [128, 512],    BF16, isOutput=False),
        'wprojT':  dp('wprojT',  [C, C],        BF16, isOutput=False),
        'bproj':   dp('bproj',   [128, 2],      F32,  isOutput=False),
        'cosq':    dp('cosq',    [128, L],      BF16, isOutput=False),
        'sinq':    dp('sinq',    [128, L],      BF16, isOutput=False),
        'rotmat':  dp('rotmat',  [128, 128],    BF16, isOutput=False),
        'ident':   dp('ident',   [128, 128],    BF16, isOutput=False),
        'diagw':   dp('diagw',   [2, 128, 25 * 128], BF16, isOutput=False),
        'blepe':   dp('blepe',   [128, 2],      F32,  isOutput=False),
        'g1bc':    dp('g1bc',    [128, C],      F32,  isOutput=False),
        'b1bc':    dp('b1bc',    [128, C],      F32,  isOutput=False),
        'out':     dp('out',     [BPC, C, L],   F32,  isOutput=True),
    }
    with tile.TileContext(nc) as tc:
        _emit(tc, io)
    nc.compile()
    return nc


def _emit(tc, io):
    with (tc.tile_pool(name="cw", bufs=1) as cw,
          tc.tile_pool(name="sb", bufs=2) as sb,
          tc.tile_pool(name="pp", bufs=2, space="PSUM") as pp):
        _emit_body(tc, io, cw, sb, pp)


def _emit_body(tc, io, cw, sb, pp):
    nc = tc.nc
    dma = nc.sync.dma_start

    # ------------------------------------------------------------------
    # persistent constants
    # ------------------------------------------------------------------
    def cload(name, dtype):
        src = io[name]
        t = cw.tile(list(src.shape), dtype, name=f"c_{name}")
        dma(out=t, in_=src[:])
        return t

    def cload2(name, dtype, cols):
        ts = [cw.tile([128, cols], dtype, name=f"c_{name}{i}") for i in range(2)]
        for i in range(2):
            dma(out=ts[i], in_=io[name][i * 128:(i + 1) * 128, :])
        return ts

    wqkT = cload2('wqkT', BF16, 512)
    # x tiles loaded up front (with the q/k weights they gate the first conv)
    xt_a = {}
    for b_ in range(BPC):
        xt_a[b_] = []
        for ct_ in range(2):
            x_t = sb.tile([128, L], BF16, name=f"x_b{b_}c{ct_}", tag="xt",
                          bufs=4)
            dma(out=x_t, in_=io['x2'][b_, ct_ * 128:(ct_ + 1) * 128, :])
            xt_a[b_].append(x_t)
    # remaining constants in order of first use
    bqk = cload('bqk', F32)
    rotmat = cload('rotmat', BF16)
    cosq = cload('cosq', BF16); sinq = cload('sinq', BF16)
    rhsvg = cload2('rhsvg', BF16, 512)
    bvgbc = cload('bvgbc', BF16)
    wvT = cload2('wvT', BF16, C)
    bv = cload('bv', F32)
    ident = cload('ident', BF16)
    blepe = cload('blepe', F32)
    g1bc = cload('g1bc', F32); b1bc = cload('b1bc', F32)
    wprojT = cload2('wprojT', BF16, C)
    bproj = cload('bproj', F32)

    epsc = cw.tile([128, 1], F32, name="epsc")
    nc.gpsimd.memset(epsc, LN_EPS)
    esshift = cw.tile([128, 1], F32, name="esshift")
    nc.gpsimd.memset(esshift, ES_SHIFT)

    # lepe diagonal stationaries diag[ct][:, tap, :] = diag(w5[ct][:, tap])
    # are host-built and DMA'd (DMA is nearly idle; frees ~7us of DVE)
    diag = [cw.tile([128, 25, 128], BF16, name=f"diag{i}") for i in range(2)]
    for i in range(2):
        dma(out=diag[i].rearrange("p a b -> p (a b)"), in_=io['diagw'][i])

    # q/k head packing: 4 heads per 128-tile at partition offsets 0/32/64/96.
    # Tiles: 0 = q heads 0-3, 1 = q heads 4-7, 2 = k heads 0-3, 3 = k heads 4-7
    # The attention SCALE is folded into the exp ACT's scale argument, so q
    # and k share one (unscaled) cos/sin table pair.
    sel_cos = [cosq, cosq, cosq, cosq]
    sel_sin = [sinq, sinq, sinq, sinq]

    qk_a, vT_a, gate_a, vpad_a, lepe_a, y_a = ({} for _ in range(6))
    es_a = {}

    def emit_qk_conv(b, m):
        # one M-tile of the q/k 1x1 conv; 4 M-tiles: q0 q1 k0 k1
        if b not in qk_a:
            qk_a[b] = {}
        qk_t = sb.tile([128, L], BF16, name=f"qk_b{b}m{m}", tag="qk", bufs=8)
        for n in range(2):
            ps = pp.tile([128, 512], F32, name="ps_mm", tag="ps_mm")
            for kc in range(2):
                nc.tensor.matmul(
                    ps, wqkT[kc][:, m * 128:(m + 1) * 128],
                    xt_a[b][kc][:, n * 512:(n + 1) * 512],
                    start=(kc == 0), stop=(kc == 1))
            nc.vector.tensor_scalar_add(
                out=qk_t[:, n * 512:(n + 1) * 512], in0=ps,
                scalar1=bqk[:, m:m + 1])
        qk_a[b][m] = qk_t

    def emit_rope(b, t):
        # theta shift on q/k tile t, in [d, l] layout
        for n in range(2):
            sl = slice(n * 512, (n + 1) * 512)
            ps = pp.tile([128, 512], F32, name="ps_rot", tag="ps_mm")
            nc.tensor.matmul(ps, rotmat, qk_a[b][t][:, sl],
                             start=True, stop=True)
            gtmp = sb.tile([128, 512], BF16, name="rope_g", tag="rope_g", bufs=2)
            nc.gpsimd.tensor_mul(out=gtmp, in0=qk_a[b][t][:, sl],
                                 in1=sel_cos[t][:, sl])
            vtmp = sb.tile([128, 512], BF16, name="rope_v", tag="rope_v", bufs=2)
            nc.vector.tensor_mul(out=vtmp, in0=ps, in1=sel_sin[t][:, sl])
            nc.vector.tensor_add(out=qk_a[b][t][:, sl], in0=gtmp, in1=vtmp)

    def emit_vg(b, lt):
        # x-stationary pass: v^T and gate^T in [l, .] layout.
        # gate = g*(1+tanh(g/2)) = 2*silu(g); tanh shares the Exp ACT table
        # set so no table reloads between gate and softmax.
        if lt == 0:
            vT_a[b] = []
            gate_a[b] = []
        ps = pp.tile([128, 512], F32, name="ps_vg", tag="ps_mm")
        for kc in range(2):
            nc.tensor.matmul(
                ps, xt_a[b][kc][:, lt * 128:(lt + 1) * 128], rhsvg[kc],
                start=(kc == 0), stop=(kc == 1))
        vT_t = sb.tile([128, NH, HD + 1], BF16, name=f"vT_b{b}l{lt}",
                       tag="vT", bufs=16)
        nc.gpsimd.memset(vT_t[:, :, HD:HD + 1], 1.0)
        nc.vector.tensor_tensor(
            out=vT_t[:, :, 0:HD],
            in0=ps[:, 0:256].rearrange("p (h d) -> p h d", h=NH),
            in1=bvgbc[:, 0:256].rearrange("p (h d) -> p h d", h=NH),
            op=AluOpType.add)
        vT_a[b].append(vT_t)
        gate_t = sb.tile([128, C], BF16, name=f"gate_b{b}l{lt}",
                         tag="gate", bufs=16)
        gb = sb.tile([128, C], F32, name="gb", tag="gb", bufs=4)
        nc.vector.tensor_add(out=gb, in0=ps[:, 256:512], in1=bvgbc[:, 256:512])
        tnt = sb.tile([128, C], F32, name="tnt", tag="tnt", bufs=3)
        nc.scalar.activation(out=tnt, in_=gb, func=AF.Tanh, scale=0.5)
        wt_ = sb.tile([128, C], F32, name="wt_", tag="wt_", bufs=2)
        nc.gpsimd.tensor_mul(out=wt_, in0=gb, in1=tnt)
        nc.gpsimd.tensor_add(out=gate_t, in0=wt_, in1=gb)
        gate_a[b].append(gate_t)

    def emit_vcl(b, ct, n):
        # v in [c, l] (for lepe), into zero-padded image tiles
        if ct == 0 and n == 0:
            vpad = []
            for c2 in range(2):
                vp = sb.tile([128, 36, 36], BF16, name=f"vpad_b{b}c{c2}",
                             tag="vpad", bufs=4)
                nc.gpsimd.memset(vp, 0.0)
                vpad.append(vp)
            vpad_a[b] = vpad
        ps = pp.tile([128, 512], F32, name="ps_vcl", tag="ps_mm")
        for kc in range(2):
            nc.tensor.matmul(
                ps, wvT[kc][:, ct * 128:(ct + 1) * 128],
                xt_a[b][kc][:, n * 512:(n + 1) * 512],
                start=(kc == 0), stop=(kc == 1))
        nc.vector.tensor_scalar_add(
            out=vpad_a[b][ct][:, 2 + n * 16:2 + (n + 1) * 16, 2:34],
            in0=ps.rearrange("p (h w) -> p h w", h=16),
            scalar1=bv[:, ct:ct + 1])

    def emit_lepe(b, ct, half):
        # one half of the lepe depthwise conv: 25 chained diag matmuls
        if ct == 0 and half == 0:
            lepe_a[b] = [sb.tile([128, L], BF16, name=f"lepe_b{b}c{c2}",
                                 tag="lepe", bufs=4) for c2 in range(2)]
        vp = vpad_a[b][ct]
        ps = pp.tile([128, 512], F32, name="ps_lepe", tag="ps_mm")
        for tap in range(25):
            dy, dx = tap // 5, tap % 5
            rhs = vp[:, dy + half * 16:dy + half * 16 + 16, dx:dx + 32]
            nc.tensor.matmul(ps, diag[ct][:, tap, :], rhs,
                             start=(tap == 0), stop=(tap == 24))
        nc.vector.tensor_scalar_add(
            out=lepe_a[b][ct][:, half * 512:(half + 1) * 512], in0=ps,
            scalar1=blepe[:, ct:ct + 1])

    def emit_scores_group(b, hg, mt, n):
        # 4 heads of group hg computed concurrently via row-tiled matmuls
        # (K=32 each, tile_position=(32i, 0)) into one 4-bank PSUM tile,
        # evacuated by a single merged exp ACT (N=2048) on ScalarE.
        if (b, hg) not in es_a:
            es_a[(b, hg)] = {}
        qt = qk_a[b][hg]
        kt = qk_a[b][2 + hg]
        ps4 = pp.tile([128, 4, 512], F32, name="ps4", tag="ps4", bufs=1)
        for i in range(4):
            nc.tensor.matmul(
                ps4[:, i, :],
                kt[32 * i:32 * i + 32, mt * 128:(mt + 1) * 128],
                qt[32 * i:32 * i + 32, n * 512:(n + 1) * 512],
                start=True, stop=True, tile_position=(32 * i, 0))
        es_t = sb.tile([128, 4, 512], ES_DT, name=f"es_b{b}g{hg}m{mt}n{n}",
                       tag="es", bufs=ES_BUFS)
        nc.scalar.activation(out=es_t, in_=ps4, func=AF.Exp, scale=SCALE,
                             bias=esshift)
        es_a[(b, hg)][(mt, n)] = es_t

    def emit_pv_unit(b, hg, lt):
        # PV for all 4 heads of group hg at l-tile lt: 4 chains of 8
        # accumulating matmuls into one PSUM bank, then softmax-normalize
        # directly from PSUM into y[lt] columns [hg*128, hg*128+128).
        if hg == 0 and lt == 0:
            y_a[b] = [sb.tile([128, C], BF16, name=f"y_b{b}l{l2}", tag="y",
                              bufs=8) for l2 in range(8)]
        es = es_a[(b, hg)]
        ps_pv = pp.tile([128, 4, HD + 1], F32, name="ps_pv", tag="ps_pv",
                        bufs=2)
        for i in range(4):
            h = hg * 4 + i
            for mc in range(8):
                lhsT = es[(mc, lt // 4)][:, i,
                                         (lt % 4) * 128:(lt % 4) * 128 + 128]
                nc.tensor.matmul(ps_pv[:, i, :], lhsT, vT_a[b][mc][:, h, :],
                                 start=(mc == 0), stop=(mc == 7))
        rcp4 = sb.tile([128, 4], F32, name="rcp4", tag="rcp4", bufs=8)
        nc.vector.reciprocal(out=rcp4, in_=ps_pv[:, :, HD])
        nc.vector.tensor_tensor(
            out=y_a[b][lt][:, hg * 128:(hg + 1) * 128].rearrange(
                "p (h d) -> p h d", h=4),
            in0=ps_pv[:, :, 0:HD],
            in1=rcp4.rearrange("p (h o) -> p h o", o=1).broadcast_to(
                [128, 4, HD]),
            op=AluOpType.mult)

    # ---------------- LN helpers ----------------
    st1_a, ln1_a, st2_a, ln2_a = {}, {}, {}, {}

    def emit_rsqrt(out_ap, var_ap):
        # out = (var + eps)^-0.5 on VectorE: quake seed + 2 Newton steps.
        # (keeps ScalarE pinned to the exp/tanh table set -- no reloads)
        g = out_ap.shape[-1]
        vp = sb.tile([128, 8], F32, name="rsq_vp", tag="rsq_vp", bufs=4)
        t = sb.tile([128, 8], F32, name="rsq_t", tag="rsq_t", bufs=4)
        vps = vp[:, 0:g]; ts = t[:, 0:g]
        nc.vector.tensor_scalar_add(out=vps, in0=var_ap, scalar1=epsc)
        nc.vector.tensor_scalar(
            out=ts.bitcast(I32), in0=vps.bitcast(I32), scalar1=1,
            scalar2=None, op0=AluOpType.logical_shift_right)
        nc.vector.tensor_scalar(
            out=out_ap.bitcast(I32), in0=ts.bitcast(I32), scalar1=-1,
            scalar2=0x5f3759df, op0=AluOpType.mult, op1=AluOpType.add)
        nc.vector.tensor_scalar_mul(out=vps, in0=vps, scalar1=0.5)
        for _ in range(2):
            # two Newton steps: ~5e-6 max rel err from the quake seed
            nc.vector.tensor_mul(out=ts, in0=out_ap, in1=out_ap)
            nc.vector.tensor_mul(out=ts, in0=ts, in1=vps)
            nc.vector.tensor_scalar(out=ts, in0=ts, scalar1=-1.0, scalar2=1.5,
                                    op0=AluOpType.mult, op1=AluOpType.add)
            nc.vector.tensor_mul(out=out_ap, in0=out_ap, in1=ts)

    def _ln2_aggr(b, g):
        # aggregate LN2 stats for l-tiles [4g, 4g+4)
        if g == 0:
            ln2_a[b] = (
                sb.tile([128, 8, 2], F32, name=f"mv8b_b{b}", tag="mv8", bufs=4),
                sb.tile([128, 8], F32, name=f"rs8b_b{b}", tag="rs8", bufs=4))
        mv8b, rs8b = ln2_a[b]
        gs = slice(g * 4, g * 4 + 4)
        for lt in range(g * 4, g * 4 + 4):
            nc.vector.bn_aggr(out=mv8b[:, lt, :], in_=st2_a[b][:, lt, :])
        emit_rsqrt(rs8b[:, gs], mv8b[:, gs, 1])

    def _ln1_apply(b, lt):
        y = y_a[b]
        mv8, rs8 = ln1_a[b]
        if lt == 0:
            st2_a[b] = sb.tile([128, 8, 6], F32, name=f"st8b_b{b}", tag="st8",
                               bufs=4)
        nc.vector.tensor_scalar(
            out=y[lt], in0=y[lt], scalar1=mv8[:, lt, 0:1],
            scalar2=rs8[:, lt:lt + 1],
            op0=AluOpType.subtract, op1=AluOpType.mult)
        t1 = sb.tile([128, C], F32, name="ln_t1", tag="ln_t1", bufs=2)
        nc.gpsimd.tensor_mul(out=t1, in0=y[lt], in1=g1bc)
        nc.gpsimd.tensor_add(out=t1, in0=t1, in1=b1bc)
        nc.vector.tensor_mul(out=y[lt], in0=t1, in1=gate_a[b][lt])
        nc.vector.bn_stats(out=st2_a[b][:, lt, :], in_=y[lt])

    def emit_tail_lt(b, lt):
        # runs right after pv_unit(b, 1, lt): lepe transpose-add, LN1 stats;
        # LN1 normalize+gate pipelined into the remaining attention work.
        y = y_a[b]
        if lt == 0:
            st1_a[b] = sb.tile([128, 8, 6], F32, name=f"st8_b{b}", tag="st8",
                               bufs=4)
        for ct in range(2):
            ps = pp.tile([128, 128], BF16, name="ps_tr", tag="ps_mm")
            nc.tensor.transpose(ps, lepe_a[b][ct][:, lt * 128:(lt + 1) * 128],
                                ident)
            sl = slice(ct * 128, (ct + 1) * 128)
            nc.vector.tensor_add(out=y[lt][:, sl], in0=y[lt][:, sl], in1=ps)
        nc.vector.bn_stats(out=st1_a[b][:, lt, :], in_=y[lt])
        if lt == 3 or lt == 7:
            g = lt // 4
            gs = slice(g * 4, g * 4 + 4)
            if g == 0:
                ln1_a[b] = (
                    sb.tile([128, 8, 2], F32, name=f"mv8_b{b}", tag="mv8",
                            bufs=4),
                    sb.tile([128, 8], F32, name=f"rs8_b{b}", tag="rs8",
                            bufs=4))
            mv8, rs8 = ln1_a[b]
            for l2 in range(g * 4, g * 4 + 4):
                nc.vector.bn_aggr(out=mv8[:, l2, :], in_=st1_a[b][:, l2, :])
            emit_rsqrt(rs8[:, gs], mv8[:, gs, 1])
            for l2 in range(g * 4, g * 4 + 4):
                _ln1_apply(b, l2)
            # the whole LN2+proj pipeline for this half runs here too: its
            # l-tiles are final, and proj's n-chunk only reads this half
            _ln2_aggr(b, g)
            emit_ln_half(b, g)
            for mt in range(2):
                emit_proj(b, mt, g)

    y2T_a = {}

    def emit_ln_half(b, g):
        # LN2 normalize + transpose to [c, l] for l-tiles [4g, 4g+4)
        y = y_a[b]
        mv8b, rs8b = ln2_a[b]
        if g == 0:
            y2T_a[b] = [sb.tile([128, L], BF16, name=f"y2T_b{b}c{ct2}",
                                tag="y2T", bufs=4) for ct2 in range(2)]
        y2T = y2T_a[b]
        for lt in range(g * 4, g * 4 + 4):
            y2b = sb.tile([128, C], BF16, name="y2b", tag="y2b", bufs=8)
            nc.vector.tensor_scalar(
                out=y2b, in0=y[lt], scalar1=mv8b[:, lt, 0:1],
                scalar2=rs8b[:, lt:lt + 1],
                op0=AluOpType.subtract, op1=AluOpType.mult)
            for ct in range(2):
                ps = pp.tile([128, 128], BF16, name="ps_tr2", tag="ps_mm")
                nc.tensor.transpose(
                    ps, y2b[:, ct * 128:(ct + 1) * 128], ident)
                nc.vector.tensor_copy(
                    out=y2T[ct][:, lt * 128:(lt + 1) * 128], in_=ps)

    def emit_proj(b, mt, n):
        y2T = y2T_a[b]
        o_t = sb.tile([128, 512], F32, name=f"o_b{b}m{mt}n{n}", tag="osb",
                      bufs=2)
        ps = pp.tile([128, 512], F32, name="ps_proj", tag="ps_mm")
        for kc in range(2):
            nc.tensor.matmul(
                ps, wprojT[kc][:, mt * 128:(mt + 1) * 128],
                y2T[kc][:, n * 512:(n + 1) * 512],
                start=(kc == 0), stop=(kc == 1))
        nc.vector.tensor_scalar_add(
            out=o_t, in0=ps, scalar1=bproj[:, mt:mt + 1])
        dma(out=io['out'][b, mt * 128:(mt + 1) * 128,
                          n * 512:(n + 1) * 512],
            in_=o_t)

    # ---------------- schedule ----------------
    def sc_phase(b, hg, fillers, pv0=None, pv1=None):
        # 16 scores groups.  pv0(lt) is emitted after group j=lt (j<8) --
        # the previous head-group's PV+normalize; pv1(k) after groups
        # j=8,10,12,14 -- THIS head-group's PV for l-tiles 0-3 (their es
        # n=0 tiles are complete after group j=7).  Other fillers are
        # spread evenly to keep the in-order PE stream fed during ACTs.
        fi = 0
        groups = [(n, mt) for n in (0, 1) for mt in range(8)]
        for j, (n, mt) in enumerate(groups):
            emit_scores_group(b, hg, mt, n)
            if pv0 is not None and j < 8:
                pv0(j)
            if pv1 is not None and j >= 8 and (j - 8) % 2 == 0:
                pv1((j - 8) // 2)
            want = (j + 1) * len(fillers) // 16
            while fi < want:
                fillers[fi](); fi += 1

    # minimal pre-critical-path for batch 0: only head-group-0 q/k tiles
    # (m=0 q, m=2 k) and their rope; everything else runs as scores fillers.
    emit_qk_conv(0, 0)
    emit_qk_conv(0, 2)
    emit_rope(0, 0)
    emit_rope(0, 2)

    F0 = [lambda: emit_qk_conv(0, 1), lambda: emit_qk_conv(0, 3),
          lambda: emit_rope(0, 1), lambda: emit_rope(0, 3)]
    for lt in range(8):
        F0.append(lambda lt=lt: emit_vg(0, lt))
    for ct in range(2):
        for n in range(2):
            F0.append(lambda ct=ct, n=n: emit_vcl(0, ct, n))
    for ct in range(2):
        for half in range(2):
            F0.append(lambda ct=ct, half=half: emit_lepe(0, ct, half))
    sc_phase(0, 0, F0)

    F0b = []
    for m in (0, 2, 1, 3):
        F0b.append(lambda m=m: emit_qk_conv(1, m))
    for t in (0, 2, 1, 3):
        F0b.append(lambda t=t: emit_rope(1, t))
    for ct in range(2):
        for n in range(2):
            F0b.append(lambda ct=ct, n=n: emit_vcl(1, ct, n))
    sc_phase(0, 1, F0b,
             pv0=lambda lt: emit_pv_unit(0, 0, lt),
             pv1=lambda k: (emit_pv_unit(0, 1, k), emit_tail_lt(0, k)))
    for lt in range(4, 8):
        emit_pv_unit(0, 1, lt)
        emit_tail_lt(0, lt)

    # ---- batch 1 ----
    F1 = []
    for lt in range(8):
        F1.append(lambda lt=lt: emit_vg(1, lt))
    for ct in range(2):
        for half in range(2):
            F1.append(lambda ct=ct, half=half: emit_lepe(1, ct, half))
    sc_phase(1, 0, F1)
    sc_phase(1, 1, [],
             pv0=lambda lt: emit_pv_unit(1, 0, lt),
             pv1=lambda k: (emit_pv_unit(1, 1, k), emit_tail_lt(1, k)))
    for lt in range(4, 8):
        emit_pv_unit(1, 1, lt)
        emit_tail_lt(1, lt)


# ----------------------------------------------------------------------
# host side
# ----------------------------------------------------------------------
def host_prep(inp):
    f32 = np.float32
    bf = lambda a: np.ascontiguousarray(a).astype(NPBF)
    p = {}
    w_qkv = np.asarray(inp['w_qkv'], f32)
    b_qkv = np.asarray(inp['b_qkv'], f32)
    # q/k weights with 4-heads-per-tile packing: head h -> tile h//4,
    # partition offset 32*(h%4); k block starts at column 256.
    wqk_pad = np.zeros((C, 512), f32)
    bqk_pad = np.zeros(512, f32)
    for h in range(NH):
        dst = (h // 4) * 128 + (h % 4) * 32
        wqk_pad[:, dst:dst + 32] = w_qkv[h * 32:(h + 1) * 32].T
        wqk_pad[:, 256 + dst:256 + dst + 32] = \
            w_qkv[256 + h * 32:256 + (h + 1) * 32].T
        bqk_pad[dst:dst + 32] = b_qkv[h * 32:(h + 1) * 32]
        bqk_pad[256 + dst:256 + dst + 32] = b_qkv[256 + h * 32:256 + (h + 1) * 32]
    p['wqkT'] = bf(wqk_pad)
    p['bqk'] = np.ascontiguousarray(bqk_pad.reshape(4, 128).T)
    p['wvT'] = bf(w_qkv[512:].T)
    p['bv'] = np.ascontiguousarray(b_qkv[512:].reshape(2, 128).T)
    s = np.asarray(inp['bn_gamma'], f32) / np.sqrt(np.float32(1.0) + f32(BN_EPS))
    wg = np.asarray(inp['w_gate'], f32) * s[:, None]
    bg = np.asarray(inp['b_gate'], f32) * s + np.asarray(inp['bn_beta'], f32)
    p['rhsvg'] = bf(np.concatenate([w_qkv[512:].T, wg.T], axis=1))
    p['bvgbc'] = bf(np.tile(np.concatenate([b_qkv[512:], bg])[None, :], (128, 1)))
    wp = np.asarray(inp['w_proj'], f32) * np.asarray(inp['ln_gamma'], f32)[None, :]
    bp = (np.asarray(inp['b_proj'], f32)
          + np.asarray(inp['w_proj'], f32) @ np.asarray(inp['ln_beta'], f32))
    p['wprojT'] = bf(wp.T)
    p['bproj'] = np.ascontiguousarray(bp.reshape(2, 128).T)
    cosl = np.asarray(inp['cos'], f32).reshape(L, HD).T
    sinl = np.asarray(inp['sin'], f32).reshape(L, HD).T
    p['cosq'] = bf(np.tile(cosl, (4, 1)))
    p['sinq'] = bf(np.tile(sinl, (4, 1)))
    R = np.zeros((128, 128), f32)
    for i in range(64):
        R[2 * i + 1, 2 * i] = -1.0
        R[2 * i, 2 * i + 1] = 1.0
    p['rotmat'] = bf(R)
    p['ident'] = bf(np.eye(128, dtype=f32))
    # diag[ct, :, tap*128:(tap+1)*128] = diag(w5[ct, :, tap])
    w5 = np.asarray(inp['w_lepe'], f32).reshape(2, 128, 25)
    dw = np.zeros((2, 128, 25 * 128), f32)
    idx = np.arange(128)
    for ct in range(2):
        for tap in range(25):
            dw[ct, idx, tap * 128 + idx] = w5[ct, :, tap]
    p['diagw'] = bf(dw)
    p['blepe'] = np.ascontiguousarray(
        np.asarray(inp['b_lepe'], f32).reshape(2, 128).T)
    # gate is computed as g*(1+tanh(g/2)) = 2*silu(g); the 0.5 is folded here
    p['g1bc'] = np.tile(0.5 * np.asarray(inp['norm_gamma'], f32)[None, :], (128, 1))
    p['b1bc'] = np.tile(0.5 * np.asarray(inp['norm_beta'], f32)[None, :], (128, 1))
    return p


_NC = None


def _get_nc():
    global _NC
    if _NC is None:
        _NC = build_program()
    return _NC


def make_in_maps(inputs):
    p = host_prep(inputs)
    x = np.asarray(inputs['x'], np.float32).reshape(B, C, L)
    in_maps = []
    for i in range(NCORES):
        m = dict(p)
        m['x2'] = np.ascontiguousarray(x[i * BPC:(i + 1) * BPC]).astype(NPBF)
        in_maps.append(m)
    return in_maps


def kernel(**inputs):
    from concourse.bass_utils import run_bass_kernel_spmd
    nc = _get_nc()
    in_maps = make_in_maps(inputs)
    res = run_bass_kernel_spmd(nc, in_maps, core_ids=list(range(NCORES)))
    outs = [np.asarray(res.results[i]['out'], np.float32).reshape(BPC, C, H, W)
            for i in range(NCORES)]
    return np.concatenate(outs, axis=0)


# revision 41
# speedup vs baseline: 1.0179x; 1.0179x over previous
"""Trainium2 Bass kernel for nn_Attention_25572235280790.

Dense attention block (B=16, C=256, H=W=32, NH=8, HD=32) with RoPE-style theta
shift, LePE depthwise 5x5 conv, BN+SiLU gate, channel LayerNorms and 1x1 convs.

Sharding: data-parallel over batch across 8 NeuronCores (2 batches/core), no
collectives.  Inside each core everything is computed in two layouts:
  - [c, l]  (channels on partitions)  for the 1x1 convs / scores / lepe
  - [l, c]  (spatial on partitions)   for softmax-normalize / layernorms / gate

v2 layout: q/k packed 4 heads per 128-partition tile (offsets 0/32/64/96).
Scores S^T[m, l] for the 4 heads of a group are computed CONCURRENTLY with
row-tiled matmuls (tile_position=(32i, 0), K=32 each) into one 4-bank PSUM
tile, and evacuated by a single merged exp ACT (N=2048) on ScalarE -- the
bottleneck engine.  PV accumulates all 4 heads of an l-tile into one PSUM
bank; softmax-normalize reads PSUM directly (no copy), dividing by the
ones-augmented V column sum.  LN rsqrt is computed on VectorE via the quake
bit-trick + 2 Newton steps, so ScalarE never leaves the exp/tanh table set.
"""

import numpy as np
import ml_dtypes

import concourse.bass as bass
import concourse.tile as tile
from concourse import bacc
from concourse import mybir
from concourse.alu_op_type import AluOpType

B, C, H, W = 16, 256, 32, 32
NH, HD = 8, 32
SCALE = HD ** -0.5
LN_EPS = 1e-6
BN_EPS = 1e-5
L = H * W
NCORES = 8
BPC = B // NCORES          # batches per core
AF = mybir.ActivationFunctionType
F32 = mybir.dt.float32
BF16 = mybir.dt.bfloat16
I32 = mybir.dt.int32

NPBF = ml_dtypes.bfloat16
# es tiles are fp8-e3m4: softmax is shift-invariant, so exp(s*SCALE - 0.5)
# keeps the observed range [0.057, 20.9] inside e3m4's [~0.008, 15.5] while
# e3m4's 4 mantissa bits halve the quantization error of e4m3.
ES_DT = mybir.dt.float8e3
ES_SHIFT = -0.5
ES_BUFS = 40


def build_program():
    nc = bacc.Bacc()
    dp = nc.declare_dram_parameter
    io = {
        'x2':      dp('x2',      [BPC, C, L],   BF16, isOutput=False),
        'wqkT':    dp('wqkT',    [C, 512],      BF16, isOutput=False),
        'bqk':     dp('bqk',     [128, 4],      F32,  isOutput=False),
        'wvT':     dp('wvT',     [C, C],        BF16, isOutput=False),
        'bv':      dp('bv',      [128, 2],      F32,  isOutput=False),
        'rhsvg':   dp('rhsvg',   [C, 512],      BF16, isOutput=False),
        'bvgbc':   dp('bvgbc',   [128, 512],    BF16, isOutput=False),
        'wprojT':  dp('wprojT',  [C, C],        BF16, isOutput=False),
        'bproj':   dp('bproj',   [128, 2],      F32,  isOutput=False),
        'cosq':    dp('cosq',    [128, L],      BF16, isOutput=False),
        'sinq':    dp('sinq',    [128, L],      BF16, isOutput=False),
        'rotmat':  dp('rotmat',  [128, 128],    BF16, isOutput=False),
        'ident':   dp('ident',   [128, 128],    BF16, isOutput=False),
        'diagw':   dp('diagw',   [2, 128, 25 * 128], BF16, isOutput=False),
        'blepe':   dp('blepe',   [128, 2],      F32,  isOutput=False),
        'g1bc':    dp('g1bc',    [128, C],      F32,  isOutput=False),
        'b1bc':    dp('b1bc',    [128, C],      F32,  isOutput=False),
        'out':     dp('out',     [BPC, C, L],   F32,  isOutput=True),
    }
    with tile.TileContext(nc) as tc:
        _emit(tc, io)
    nc.compile()
    return nc


def _emit(tc, io):
    with (tc.tile_pool(name="cw", bufs=1) as cw,
          tc.tile_pool(name="sb", bufs=2) as sb,
          tc.tile_pool(name="pp", bufs=2, space="PSUM") as pp):
        _emit_body(tc, io, cw, sb, pp)


def _emit_body(tc, io, cw, sb, pp):
    nc = tc.nc
    dma = nc.sync.dma_start

    # ------------------------------------------------------------------
    # persistent constants
    # ------------------------------------------------------------------
    def cload(name, dtype):
        src = io[name]
        t = cw.tile(list(src.shape), dtype, name=f"c_{name}")
        dma(out=t, in_=src[:])
        return t

    def cload2(name, dtype, cols):
        ts = [cw.tile([128, cols], dtype, name=f"c_{name}{i}") for i in range(2)]
        for i in range(2):
            dma(out=ts[i], in_=io[name][i * 128:(i + 1) * 128, :])
        return ts

    wqkT = cload2('wqkT', BF16, 512)
    # x tiles loaded up front (with the q/k weights they gate the first conv)
    xt_a = {}
    for b_ in range(BPC):
        xt_a[b_] = []
        for ct_ in range(2):
            x_t = sb.tile([128, L], BF16, name=f"x_b{b_}c{ct_}", tag="xt",
                          bufs=4)
            dma(out=x_t, in_=io['x2'][b_, ct_ * 128:(ct_ + 1) * 128, :])
            xt_a[b_].append(x_t)
    # remaining constants in order of first use
    bqk = cload('bqk', F32)
    rotmat = cload('rotmat', BF16)
    cosq = cload('cosq', BF16); sinq = cload('sinq', BF16)
    rhsvg = cload2('rhsvg', BF16, 512)
    bvgbc = cload('bvgbc', BF16)
    wvT = cload2('wvT', BF16, C)
    bv = cload('bv', F32)
    ident = cload('ident', BF16)
    blepe = cload('blepe', F32)
    g1bc = cload('g1bc', F32); b1bc = cload('b1bc', F32)
    wprojT = cload2('wprojT', BF16, C)
    bproj = cload('bproj', F32)

    epsc = cw.tile([128, 1], F32, name="epsc")
    nc.gpsimd.memset(epsc, LN_EPS)
    esshift = cw.tile([128, 1], F32, name="esshift")
    nc.gpsimd.memset(esshift, ES_SHIFT)

    # lepe diagonal stationaries diag[ct][:, tap, :] = diag(w5[ct][:, tap])
    # are host-built and DMA'd (DMA is nearly idle; frees ~7us of DVE)
    diag = [cw.tile([128, 25, 128], BF16, name=f"diag{i}") for i in range(2)]
    for i in range(2):
        dma(out=diag[i].rearrange("p a b -> p (a b)"), in_=io['diagw'][i])

    # q/k head packing: 4 heads per 128-tile at partition offsets 0/32/64/96.
    # Tiles: 0 = q heads 0-3, 1 = q heads 4-7, 2 = k heads 0-3, 3 = k heads 4-7
    # The attention SCALE is folded into the exp ACT's scale argument, so q
    # and k share one (unscaled) cos/sin table pair.
    sel_cos = [cosq, cosq, cosq, cosq]
    sel_sin = [sinq, sinq, sinq, sinq]

    qk_a, vT_a, gate_a, vpad_a, lepe_a, y_a = ({} for _ in range(6))
    es_a = {}

    def emit_qk_conv(b, m, scalar_evac=False):
        # one M-tile of the q/k 1x1 conv; 4 M-tiles: q0 q1 k0 k1.
        # scalar_evac routes the PSUM evacuation through ScalarE (idle at
        # kernel start) to keep the startup critical path off the DVE.
        if b not in qk_a:
            qk_a[b] = {}
        qk_t = sb.tile([128, L], BF16, name=f"qk_b{b}m{m}", tag="qk", bufs=8)
        for n in range(2):
            ps = pp.tile([128, 512], F32, name="ps_mm", tag="ps_mm")
            for kc in range(2):
                nc.tensor.matmul(
                    ps, wqkT[kc][:, m * 128:(m + 1) * 128],
                    xt_a[b][kc][:, n * 512:(n + 1) * 512],
                    start=(kc == 0), stop=(kc == 1))
            if scalar_evac:
                nc.scalar.activation(
                    out=qk_t[:, n * 512:(n + 1) * 512], in_=ps,
                    func=AF.Identity, bias=bqk[:, m:m + 1], scale=1.0)
            else:
                nc.vector.tensor_scalar_add(
                    out=qk_t[:, n * 512:(n + 1) * 512], in0=ps,
                    scalar1=bqk[:, m:m + 1])
        qk_a[b][m] = qk_t

    def emit_rope(b, t):
        # theta shift on q/k tile t, in [d, l] layout
        for n in range(2):
            sl = slice(n * 512, (n + 1) * 512)
            ps = pp.tile([128, 512], F32, name="ps_rot", tag="ps_mm")
            nc.tensor.matmul(ps, rotmat, qk_a[b][t][:, sl],
                             start=True, stop=True)
            gtmp = sb.tile([128, 512], BF16, name="rope_g", tag="rope_g", bufs=2)
            nc.gpsimd.tensor_mul(out=gtmp, in0=qk_a[b][t][:, sl],
                                 in1=sel_cos[t][:, sl])
            vtmp = sb.tile([128, 512], BF16, name="rope_v", tag="rope_v", bufs=2)
            nc.vector.tensor_mul(out=vtmp, in0=ps, in1=sel_sin[t][:, sl])
            nc.vector.tensor_add(out=qk_a[b][t][:, sl], in0=gtmp, in1=vtmp)

    def emit_vg(b, lt):
        # x-stationary pass: v^T and gate^T in [l, .] layout.
        # gate = g*(1+tanh(g/2)) = 2*silu(g); tanh shares the Exp ACT table
        # set so no table reloads between gate and softmax.
        if lt == 0:
            vT_a[b] = []
            gate_a[b] = []
        ps = pp.tile([128, 512], F32, name="ps_vg", tag="ps_mm")
        for kc in range(2):
            nc.tensor.matmul(
                ps, xt_a[b][kc][:, lt * 128:(lt + 1) * 128], rhsvg[kc],
                start=(kc == 0), stop=(kc == 1))
        vT_t = sb.tile([128, NH, HD + 1], BF16, name=f"vT_b{b}l{lt}",
                       tag="vT", bufs=16)
        nc.gpsimd.memset(vT_t[:, :, HD:HD + 1], 1.0)
        nc.vector.tensor_tensor(
            out=vT_t[:, :, 0:HD],
            in0=ps[:, 0:256].rearrange("p (h d) -> p h d", h=NH),
            in1=bvgbc[:, 0:256].rearrange("p (h d) -> p h d", h=NH),
            op=AluOpType.add)
        vT_a[b].append(vT_t)
        gate_t = sb.tile([128, C], BF16, name=f"gate_b{b}l{lt}",
                         tag="gate", bufs=16)
        gb = sb.tile([128, C], F32, name="gb", tag="gb", bufs=4)
        nc.vector.tensor_add(out=gb, in0=ps[:, 256:512], in1=bvgbc[:, 256:512])
        tnt = sb.tile([128, C], F32, name="tnt", tag="tnt", bufs=3)
        nc.scalar.activation(out=tnt, in_=gb, func=AF.Tanh, scale=0.5)
        wt_ = sb.tile([128, C], F32, name="wt_", tag="wt_", bufs=2)
        nc.gpsimd.tensor_mul(out=wt_, in0=gb, in1=tnt)
        nc.gpsimd.tensor_add(out=gate_t, in0=wt_, in1=gb)
        gate_a[b].append(gate_t)

    def emit_vcl(b, ct, n):
        # v in [c, l] (for lepe), into zero-padded image tiles
        if ct == 0 and n == 0:
            vpad = []
            for c2 in range(2):
                vp = sb.tile([128, 36, 36], BF16, name=f"vpad_b{b}c{c2}",
                             tag="vpad", bufs=4)
                nc.gpsimd.memset(vp, 0.0)
                vpad.append(vp)
            vpad_a[b] = vpad
        ps = pp.tile([128, 512], F32, name="ps_vcl", tag="ps_mm")
        for kc in range(2):
            nc.tensor.matmul(
                ps, wvT[kc][:, ct * 128:(ct + 1) * 128],
                xt_a[b][kc][:, n * 512:(n + 1) * 512],
                start=(kc == 0), stop=(kc == 1))
        nc.vector.tensor_scalar_add(
            out=vpad_a[b][ct][:, 2 + n * 16:2 + (n + 1) * 16, 2:34],
            in0=ps.rearrange("p (h w) -> p h w", h=16),
            scalar1=bv[:, ct:ct + 1])

    def emit_lepe(b, ct, half):
        # one half of the lepe depthwise conv: 25 chained diag matmuls
        if ct == 0 and half == 0:
            lepe_a[b] = [sb.tile([128, L], BF16, name=f"lepe_b{b}c{c2}",
                                 tag="lepe", bufs=4) for c2 in range(2)]
        vp = vpad_a[b][ct]
        ps = pp.tile([128, 512], F32, name="ps_lepe", tag="ps_mm")
        for tap in range(25):
            dy, dx = tap // 5, tap % 5
            rhs = vp[:, dy + half * 16:dy + half * 16 + 16, dx:dx + 32]
            nc.tensor.matmul(ps, diag[ct][:, tap, :], rhs,
                             start=(tap == 0), stop=(tap == 24))
        nc.vector.tensor_scalar_add(
            out=lepe_a[b][ct][:, half * 512:(half + 1) * 512], in0=ps,
            scalar1=blepe[:, ct:ct + 1])

    def emit_scores_group(b, hg, mt, n, pair):
        # one head-pair of group hg via 2 concurrent row-tiled matmuls
        # (K=32 each) into a 2-bank PSUM tile, evacuated by one merged exp
        # ACT (N=1024).  With bufs=2 the next pair's matmuls overlap this
        # pair's ACT, so neither PE nor ScalarE waits on the other.
        if (b, hg) not in es_a:
            es_a[(b, hg)] = {}
        qt = qk_a[b][hg]
        kt = qk_a[b][2 + hg]
        ps2 = pp.tile([128, 2, 512], F32, name="ps2", tag="ps2", bufs=2)
        for i2 in range(2):
            i = pair * 2 + i2
            nc.tensor.matmul(
                ps2[:, i2, :],
                kt[32 * i:32 * i + 32, mt * 128:(mt + 1) * 128],
                qt[32 * i:32 * i + 32, n * 512:(n + 1) * 512],
                start=True, stop=True, tile_position=(32 * i, 0))
        es_t = sb.tile([128, 2, 512], ES_DT,
                       name=f"es_b{b}g{hg}m{mt}n{n}p{pair}",
                       tag="es", bufs=ES_BUFS)
        nc.scalar.activation(out=es_t, in_=ps2, func=AF.Exp, scale=SCALE,
                             bias=esshift)
        es_a[(b, hg)][(mt, n, pair)] = es_t

    def emit_pv_unit(b, hg, lt):
        # PV for all 4 heads of group hg at l-tile lt: 4 chains of 8
        # accumulating matmuls into one PSUM bank, then softmax-normalize
        # directly from PSUM into y[lt] columns [hg*128, hg*128+128).
        if hg == 0 and lt == 0:
            y_a[b] = [sb.tile([128, C], BF16, name=f"y_b{b}l{l2}", tag="y",
                              bufs=8) for l2 in range(8)]
        es = es_a[(b, hg)]
        ps_pv = pp.tile([128, 4, HD + 1], F32, name="ps_pv", tag="ps_pv",
                        bufs=2)
        for i in range(4):
            h = hg * 4 + i
            for mc in range(8):
                lhsT = es[(mc, lt // 4, i // 2)][
                    :, i % 2, (lt % 4) * 128:(lt % 4) * 128 + 128]
                nc.tensor.matmul(ps_pv[:, i, :], lhsT, vT_a[b][mc][:, h, :],
                                 start=(mc == 0), stop=(mc == 7))
        rcp4 = sb.tile([128, 4], F32, name="rcp4", tag="rcp4", bufs=8)
        nc.vector.reciprocal(out=rcp4, in_=ps_pv[:, :, HD])
        nc.vector.tensor_tensor(
            out=y_a[b][lt][:, hg * 128:(hg + 1) * 128].rearrange(
                "p (h d) -> p h d", h=4),
            in0=ps_pv[:, :, 0:HD],
            in1=rcp4.rearrange("p (h o) -> p h o", o=1).broadcast_to(
                [128, 4, HD]),
            op=AluOpType.mult)

    # ---------------- LN helpers ----------------
    st1_a, ln1_a, st2_a, ln2_a = {}, {}, {}, {}

    def emit_rsqrt(out_ap, var_ap):
        # out = (var + eps)^-0.5 on VectorE: quake seed + 2 Newton steps.
        # (keeps ScalarE pinned to the exp/tanh table set -- no reloads)
        g = out_ap.shape[-1]
        vp = sb.tile([128, 8], F32, name="rsq_vp", tag="rsq_vp", bufs=4)
        t = sb.tile([128, 8], F32, name="rsq_t", tag="rsq_t", bufs=4)
        vps = vp[:, 0:g]; ts = t[:, 0:g]
        nc.vector.tensor_scalar_add(out=vps, in0=var_ap, scalar1=epsc)
        nc.vector.tensor_scalar(
            out=ts.bitcast(I32), in0=vps.bitcast(I32), scalar1=1,
            scalar2=None, op0=AluOpType.logical_shift_right)
        nc.vector.tensor_scalar(
            out=out_ap.bitcast(I32), in0=ts.bitcast(I32), scalar1=-1,
            scalar2=0x5f3759df, op0=AluOpType.mult, op1=AluOpType.add)
        nc.vector.tensor_scalar_mul(out=vps, in0=vps, scalar1=0.5)
        for _ in range(2):
            # two Newton steps: ~5e-6 max rel err from the quake seed
            nc.vector.tensor_mul(out=ts, in0=out_ap, in1=out_ap)
            nc.vector.tensor_mul(out=ts, in0=ts, in1=vps)
            nc.vector.tensor_scalar(out=ts, in0=ts, scalar1=-1.0, scalar2=1.5,
                                    op0=AluOpType.mult, op1=AluOpType.add)
            nc.vector.tensor_mul(out=out_ap, in0=out_ap, in1=ts)

    def _ln2_aggr(b, g):
        # aggregate LN2 stats for l-tiles [4g, 4g+4)
        if g == 0:
            ln2_a[b] = (
                sb.tile([128, 8, 2], F32, name=f"mv8b_b{b}", tag="mv8", bufs=4),
                sb.tile([128, 8], F32, name=f"rs8b_b{b}", tag="rs8", bufs=4))
        mv8b, rs8b = ln2_a[b]
        gs = slice(g * 4, g * 4 + 4)
        for lt in range(g * 4, g * 4 + 4):
            nc.vector.bn_aggr(out=mv8b[:, lt, :], in_=st2_a[b][:, lt, :])
        emit_rsqrt(rs8b[:, gs], mv8b[:, gs, 1])

    def _ln1_apply(b, lt):
        y = y_a[b]
        mv8, rs8 = ln1_a[b]
        if lt == 0:
            st2_a[b] = sb.tile([128, 8, 6], F32, name=f"st8b_b{b}", tag="st8",
                               bufs=4)
        nc.vector.tensor_scalar(
            out=y[lt], in0=y[lt], scalar1=mv8[:, lt, 0:1],
            scalar2=rs8[:, lt:lt + 1],
            op0=AluOpType.subtract, op1=AluOpType.mult)
        t1 = sb.tile([128, C], F32, name="ln_t1", tag="ln_t1", bufs=2)
        nc.gpsimd.tensor_mul(out=t1, in0=y[lt], in1=g1bc)
        nc.gpsimd.tensor_add(out=t1, in0=t1, in1=b1bc)
        nc.vector.tensor_mul(out=y[lt], in0=t1, in1=gate_a[b][lt])
        nc.vector.bn_stats(out=st2_a[b][:, lt, :], in_=y[lt])

    def emit_tail_lt(b, lt):
        # runs right after pv_unit(b, 1, lt): lepe transpose-add, LN1 stats;
        # LN1 normalize+gate pipelined into the remaining attention work.
        y = y_a[b]
        if lt == 0:
            st1_a[b] = sb.tile([128, 8, 6], F32, name=f"st8_b{b}", tag="st8",
                               bufs=4)
        for ct in range(2):
            ps = pp.tile([128, 128], BF16, name="ps_tr", tag="ps_mm")
            nc.tensor.transpose(ps, lepe_a[b][ct][:, lt * 128:(lt + 1) * 128],
                                ident)
            sl = slice(ct * 128, (ct + 1) * 128)
            nc.vector.tensor_add(out=y[lt][:, sl], in0=y[lt][:, sl], in1=ps)
        nc.vector.bn_stats(out=st1_a[b][:, lt, :], in_=y[lt])
        if lt == 3 or lt == 7:
            g = lt // 4
            gs = slice(g * 4, g * 4 + 4)
            if g == 0:
                ln1_a[b] = (
                    sb.tile([128, 8, 2], F32, name=f"mv8_b{b}", tag="mv8",
                            bufs=4),
                    sb.tile([128, 8], F32, name=f"rs8_b{b}", tag="rs8",
                            bufs=4))
            mv8, rs8 = ln1_a[b]
            for l2 in range(g * 4, g * 4 + 4):
                nc.vector.bn_aggr(out=mv8[:, l2, :], in_=st1_a[b][:, l2, :])
            emit_rsqrt(rs8[:, gs], mv8[:, gs, 1])
            for l2 in range(g * 4, g * 4 + 4):
                _ln1_apply(b, l2)
            # the whole LN2+proj pipeline for this half runs here too: its
            # l-tiles are final, and proj's n-chunk only reads this half
            _ln2_aggr(b, g)
            emit_ln_half(b, g)
            for mt in range(2):
                emit_proj(b, mt, g)

    y2T_a = {}

    def emit_ln_half(b, g):
        # LN2 normalize + transpose to [c, l] for l-tiles [4g, 4g+4)
        y = y_a[b]
        mv8b, rs8b = ln2_a[b]
        if g == 0:
            y2T_a[b] = [sb.tile([128, L], BF16, name=f"y2T_b{b}c{ct2}",
                                tag="y2T", bufs=4) for ct2 in range(2)]
        y2T = y2T_a[b]
        for lt in range(g * 4, g * 4 + 4):
            y2b = sb.tile([128, C], BF16, name="y2b", tag="y2b", bufs=8)
            nc.vector.tensor_scalar(
                out=y2b, in0=y[lt], scalar1=mv8b[:, lt, 0:1],
                scalar2=rs8b[:, lt:lt + 1],
                op0=AluOpType.subtract, op1=AluOpType.mult)
            for ct in range(2):
                ps = pp.tile([128, 128], BF16, name="ps_tr2", tag="ps_mm")
                nc.tensor.transpose(
                    ps, y2b[:, ct * 128:(ct + 1) * 128], ident)
                # ScalarE does this PSUM->SBUF copy: it is idle in the tail
                # and this keeps the DVE queue from stalling the transposes
                nc.scalar.copy(
                    out=y2T[ct][:, lt * 128:(lt + 1) * 128], in_=ps)

    def emit_proj(b, mt, n):
        y2T = y2T_a[b]
        o_t = sb.tile([128, 512], F32, name=f"o_b{b}m{mt}n{n}", tag="osb",
                      bufs=2)
        ps = pp.tile([128, 512], F32, name="ps_proj", tag="ps_mm")
        for kc in range(2):
            nc.tensor.matmul(
                ps, wprojT[kc][:, mt * 128:(mt + 1) * 128],
                y2T[kc][:, n * 512:(n + 1) * 512],
                start=(kc == 0), stop=(kc == 1))
        nc.vector.tensor_scalar_add(
            out=o_t, in0=ps, scalar1=bproj[:, mt:mt + 1])
        dma(out=io['out'][b, mt * 128:(mt + 1) * 128,
                          n * 512:(n + 1) * 512],
            in_=o_t)

    # ---------------- schedule ----------------
    def sc_phase(b, hg, fillers, pv0=None, pv1=None):
        # 32 head-pair scores groups.  pv0(lt) is emitted after group
        # j=2lt+1 (j<16) -- the previous head-group's PV+normalize; pv1(k)
        # after groups j=17,19,21,23 -- THIS head-group's PV for l-tiles
        # 0-3 (their es n=0 tiles are complete after group j=15).  Other
        # fillers are spread evenly to keep the PE stream fed during ACTs.
        fi = 0
        groups = [(n, mt, pr) for n in (0, 1) for mt in range(8)
                  for pr in (0, 1)]
        for j, (n, mt, pr) in enumerate(groups):
            emit_scores_group(b, hg, mt, n, pr)
            if pv0 is not None and j < 16 and j % 2 == 1:
                pv0(j // 2)
            if pv1 is not None and 16 < j < 24 and j % 2 == 1:
                pv1((j - 17) // 2)
            want = (j + 1) * len(fillers) // 32
            while fi < want:
                fillers[fi](); fi += 1

    # minimal pre-critical-path for batch 0: only head-group-0 q/k tiles
    # (m=0 q, m=2 k) and their rope; everything else runs as scores fillers.
    emit_qk_conv(0, 0, scalar_evac=True)
    emit_qk_conv(0, 2, scalar_evac=True)
    emit_rope(0, 0)
    emit_rope(0, 2)

    F0 = [lambda: emit_qk_conv(0, 1), lambda: emit_qk_conv(0, 3),
          lambda: emit_rope(0, 1), lambda: emit_rope(0, 3)]
    for lt in range(8):
        F0.append(lambda lt=lt: emit_vg(0, lt))
    for ct in range(2):
        for n in range(2):
            F0.append(lambda ct=ct, n=n: emit_vcl(0, ct, n))
    for ct in range(2):
        for half in range(2):
            F0.append(lambda ct=ct, half=half: emit_lepe(0, ct, half))
    sc_phase(0, 0, F0)

    F0b = []
    for m in (0, 2, 1, 3):
        F0b.append(lambda m=m: emit_qk_conv(1, m))
    for t in (0, 2, 1, 3):
        F0b.append(lambda t=t: emit_rope(1, t))
    for ct in range(2):
        for n in range(2):
            F0b.append(lambda ct=ct, n=n: emit_vcl(1, ct, n))
    sc_phase(0, 1, F0b,
             pv0=lambda lt: emit_pv_unit(0, 0, lt),
             pv1=lambda k: (emit_pv_unit(0, 1, k), emit_tail_lt(0, k)))
    for lt in range(4, 8):
        emit_pv_unit(0, 1, lt)
        emit_tail_lt(0, lt)

    # ---- batch 1 ----
    F1 = []
    for lt in range(8):
        F1.append(lambda lt=lt: emit_vg(1, lt))
    for ct in range(2):
        for half in range(2):
            F1.append(lambda ct=ct, half=half: emit_lepe(1, ct, half))
    sc_phase(1, 0, F1)
    sc_phase(1, 1, [],
             pv0=lambda lt: emit_pv_unit(1, 0, lt),
             pv1=lambda k: (emit_pv_unit(1, 1, k), emit_tail_lt(1, k)))
    for lt in range(4, 8):
        emit_pv_unit(1, 1, lt)
        emit_tail_lt(1, lt)


# ----------------------------------------------------------------------
# host side
# ----------------------------------------------------------------------
def host_prep(inp):
    f32 = np.float32
    bf = lambda a: np.ascontiguousarray(a).astype(NPBF)
    p = {}
    w_qkv = np.asarray(inp['w_qkv'], f32)
    b_qkv = np.asarray(inp['b_qkv'], f32)
    # q/k weights with 4-heads-per-tile packing: head h -> tile h//4,
    # partition offset 32*(h%4); k block starts at column 256.
    wqk_pad = np.zeros((C, 512), f32)
    bqk_pad = np.zeros(512, f32)
    for h in range(NH):
        dst = (h // 4) * 128 + (h % 4) * 32
        wqk_pad[:, dst:dst + 32] = w_qkv[h * 32:(h + 1) * 32].T
        wqk_pad[:, 256 + dst:256 + dst + 32] = \
            w_qkv[256 + h * 32:256 + (h + 1) * 32].T
        bqk_pad[dst:dst + 32] = b_qkv[h * 32:(h + 1) * 32]
        bqk_pad[256 + dst:256 + dst + 32] = b_qkv[256 + h * 32:256 + (h + 1) * 32]
    p['wqkT'] = bf(wqk_pad)
    p['bqk'] = np.ascontiguousarray(bqk_pad.reshape(4, 128).T)
    p['wvT'] = bf(w_qkv[512:].T)
    p['bv'] = np.ascontiguousarray(b_qkv[512:].reshape(2, 128).T)
    s = np.asarray(inp['bn_gamma'], f32) / np.sqrt(np.float32(1.0) + f32(BN_EPS))
    wg = np.asarray(inp['w_gate'], f32) * s[:, None]
    bg = np.asarray(inp['b_gate'], f32) * s + np.asarray(inp['bn_beta'], f32)
    p['rhsvg'] = bf(np.concatenate([w_qkv[512:].T, wg.T], axis=1))
    p['bvgbc'] = bf(np.tile(np.concatenate([b_qkv[512:], bg])[None, :], (128, 1)))
    wp = np.asarray(inp['w_proj'], f32) * np.asarray(inp['ln_gamma'], f32)[None, :]
    bp = (np.asarray(inp['b_proj'], f32)
          + np.asarray(inp['w_proj'], f32) @ np.asarray(inp['ln_beta'], f32))
    p['wprojT'] = bf(wp.T)
    p['bproj'] = np.ascontiguousarray(bp.reshape(2, 128).T)
    cosl = np.asarray(inp['cos'], f32).reshape(L, HD).T
    sinl = np.asarray(inp['sin'], f32).reshape(L, HD).T
    p['cosq'] = bf(np.tile(cosl, (4, 1)))
    p['sinq'] = bf(np.tile(sinl, (4, 1)))
    R = np.zeros((128, 128), f32)
    for i in range(64):
        R[2 * i + 1, 2 * i] = -1.0
        R[2 * i, 2 * i + 1] = 1.0
    p['rotmat'] = bf(R)
    p['ident'] = bf(np.eye(128, dtype=f32))
    # diag[ct, :, tap*128:(tap+1)*128] = diag(w5[ct, :, tap])
    w5 = np.asarray(inp['w_lepe'], f32).reshape(2, 128, 25)
    dw = np.zeros((2, 128, 25 * 128), f32)
    idx = np.arange(128)
    for ct in range(2):
        for tap in range(25):
            dw[ct, idx, tap * 128 + idx] = w5[ct, :, tap]
    p['diagw'] = bf(dw)
    p['blepe'] = np.ascontiguousarray(
        np.asarray(inp['b_lepe'], f32).reshape(2, 128).T)
    # gate is computed as g*(1+tanh(g/2)) = 2*silu(g); the 0.5 is folded here
    p['g1bc'] = np.tile(0.5 * np.asarray(inp['norm_gamma'], f32)[None, :], (128, 1))
    p['b1bc'] = np.tile(0.5 * np.asarray(inp['norm_beta'], f32)[None, :], (128, 1))
    return p


_NC = None


def _get_nc():
    global _NC
    if _NC is None:
        _NC = build_program()
    return _NC


def make_in_maps(inputs):
    p = host_prep(inputs)
    x = np.asarray(inputs['x'], np.float32).reshape(B, C, L)
    in_maps = []
    for i in range(NCORES):
        m = dict(p)
        m['x2'] = np.ascontiguousarray(x[i * BPC:(i + 1) * BPC]).astype(NPBF)
        in_maps.append(m)
    return in_maps


def kernel(**inputs):
    from concourse.bass_utils import run_bass_kernel_spmd
    nc = _get_nc()
    in_maps = make_in_maps(inputs)
    res = run_bass_kernel_spmd(nc, in_maps, core_ids=list(range(NCORES)))
    outs = [np.asarray(res.results[i]['out'], np.float32).reshape(BPC, C, H, W)
            for i in range(NCORES)]
    return np.concatenate(outs, axis=0)
